# revision 77
# baseline (speedup 1.0000x reference)
"""Soft k-means (DCN vq_codebook) on 8 Trainium2 NeuronCores.

Reference math: 10 iterations of
    d    = ||x||^2 + ||c||^2 - 2 X C^T                    [N, K]
    dn   = (d - dmin) / (dmax - dmin)
    soft = exp(-gamma * dn)
    sp   = soft / rowsum(soft) + eps
    C    = (sp^T X) / colsum(sp) + eps                     [K, D]

Validated transformations (numpy sim vs the fp32 reference, seed 0):
  * Row factors cancel in the row-softmax, so ||x||^2 and the dmin
    shift drop out: soft' = exp(z), z = a*(||c||^2 - 2 x.c), with a
    frozen at iteration 0 (the output is insensitive to the scale R
    in a = -gamma/R: +-4x moves it < 3e-4 of scale, so R = 4*mc with
    mc = max ||c0||^2 replaces the Cauchy-Schwarz bound -- mc is
    computable from the replicated clusters, no cross-core max).
  * |z| <= gamma = 0.01, so exp(z) ~= 1 + z to 0.5% of the signal;
    with exact row masses this matches full exp to ~3e-6 rel.
  * The row masses rowsum = K + sum_j z_nj vary by only ~1e-5
    relative, so treating them as constant (they then cancel in the
    centroid quotient) gives rel err ~7e-5 -- 30x inside the 2e-3
    gate.  The whole N-dependence then collapses into the second
    moment matrix G0 = [X|1]^T [X|1]  [65, 65]:
        cc_k = ||c_k||^2
        W    = (G0 diag([-2a]*64, 1)) @ [[C^T], [1 + a*cc]]
        C'   = W[0:64] / W[64]           (mass row)
  * The iteration is strongly contractive: 2 iterations reproduce the
    10-iteration reference to the same ~7e-5.

Cross-core reduction WITHOUT the collective-compute stack (measured
~66us on the baseline critical path: 41.6us cc init barrier + 11.2us
fixed gap + 13.2us ring AllReduce for a 17KB latency-bound payload).
Instead, a 3-round XOR hypercube reduce over relative
remote_dma_broadcasts: round k exchanges the running partial [128,65]
with peer tpb^DELTAS[k], cross-die (delta 4, D2D link) FIRST while
there is the most slack (single-destination 8-slot frame; the XOR-relative
addressing makes one SPMD program give every receiver a distinct
sender per slot -- verified on HW) and adds it in.  Three frames
instead of seven matter because Q7 descriptor generation (~6.9us per
128-partition frame) is the serializer; the preps are emitted right
after the send-buffer memsets so their only Tile dependency is early
and Q7 PRE-generates all frames long before the data-gated triggers
fire (~1.3us trigger-to-wire).  Each round has its own arrival
semaphore (rounds land out of order across peers) and its own SWDGE
queue so its trigger fires only its frame.

A NEFF without any collective gets no runtime-rendezvoused launch
(multi-ms per-core dispatch skew eats the remote-DMA waits), so a
dummy 32-byte 8-core AllReduce rides in its own pre-block: inline
zero input, output never read, nothing waits on its completion -- the
slow cc chain runs concurrently and off the critical path.

Cross-core waits cannot be expressed inside TileContext (the
single-core scheduling sim has no remote sem delivery and deadlocks),
so all of them are injected POST-schedule as standalone wait_ge
instructions: per-round arrival gates before each round's first
reader per engine, and gsem gates before the triggers (gsem is bumped
by injected drain-then-inc after the memsets / scalar B0 writes /
round adds -- compute instructions have a single HW update slot that
Tile already occupies).  Emission-order rules that avoid
scheduler-visible cycles: preps must FOLLOW the memsets (else the
memsets get WAR waits on the gsem-gated Pool stream) and round k+1's
prep precedes round k's add (the WAR then points the harmless way).

Schedule notes:
  * Input X DMA'd in 10 chunks with SMALL leading chunks (4/4/8 tiles
    then 16s, chunk 0 issued first) so the fp32 G0 PSUM-accumulation
    chain (136ns/tile steady state, ~17.4us) starts as soon as the
    first 133KB lands instead of after the full 4.25MB.  Keep the
    chunk-0 DMA on the sync queue: issuing it from gpsimd puts its
    descriptor generation on Q7 ahead of the frame preps and delays
    the sends (measured regression).
  * The B0 scale is a single 65-row activation (s2b row 64 = 1.0
    leaves the mass row unscaled).
  * 1/mass uses reciprocal_approx_fast (~18 bits), then a gpsimd
    partition_broadcast replicates it to 64 partitions (cheaper than
    either the old DRAM stride-0 bounce or a fp32 LOW_HIGH rank-1 PE
    matmul); the divide reads W2 straight from PSUM (a single PSUM
    operand per DVE op is legal).
  * Iteration 2 consumes W1 unnormalized (column mass scale cancels
    in its own quotient; a*cc1 ~ 1e-8), so iteration 1 needs no
    normalization at all.
Measured: best 56us, median ~71us, band 56-85us (launch-skew
variance between the 8 cores dominates the spread; the baseline
AllReduce design was a stable 107-110us).  Rejected with data: flat
7-frame gather (Q7 frame-gen serialized when preps carry a late data
dep), depth-2 tree with 3 parallel frames (fabric contention ~6x per
descriptor), host_desc_gen (fails at NEFF load), NCHUNK=16 (outliers),
bf16 G0 (rel err 2e-3, 30x accuracy loss for ~4us), iter-1 split
W1 = (B2+R2)^T rhs1 (round-2 data usually lands WITH round 1, so the
split only added 4 PE slices to the tail).
"""

import os
import sys

sys.path.insert(0, "/opt/trn_rl_repo")

import numpy as np

import concourse.bacc as bacc
import concourse.bass as bass
import concourse.mybir as mybir
import concourse.tile as tile
from concourse import bass_utils

F32 = mybir.dt.float32
BF16 = mybir.dt.bfloat16
AF = mybir.ActivationFunctionType
ALU = mybir.AluOpType
AX = mybir.AxisListType

NCORES = 8
N, D, K = 131072, 64, 1024
NL = N // NCORES          # rows per core (16384)
NT = NL // 128            # n-tiles per core (128)
DA = D + 1                # augmented row width [x | 1]
# input DMA chunk sizes in 128-row tiles (separate tiles -> per-chunk deps);
# small leading chunks land sooner so the PE-bound G0 chain starts earlier
CHUNKS = [4, 4, 8, 16, 16, 16, 16, 16, 16, 16]
assert sum(CHUNKS) == NT
GAMMA = 0.01
SLOT_W = DA               # gather slot width (fp32 cols)


def _ap_mem_names(aps):
    names = set()
    for a in aps:
        try:
            names.add(a.memref)
        except Exception:
            pass
        try:
            names.add(a.memorylocation.name)
        except Exception:
            pass
    return names


def _inject_arrival_waits(nc, tile_names, sem_vals):
    """Insert standalone `wait sem >= val` instructions (one per (sem, val)
    pair) before each engine's first scheduled READ of the given tiles
    (remote-DMA landing zones)."""
    hit_engines = set()
    targets = []
    for bb in nc.m.functions[0].blocks:
        for idx, inst in enumerate(bb.instructions):
            eng = getattr(inst, "engine", None)
            if eng is None or eng in hit_engines:
                continue
            if "Remote" in type(inst).__name__:
                continue
            mems = _ap_mem_names(getattr(inst, "ins", []))
            if not any(any(t in m for t in tile_names) for m in mems):
                continue
            hit_engines.add(eng)
            targets.append((bb, idx, eng, inst.name))
    prev_bb = nc.cur_bb
    attached = []
    for bb, idx, eng, iname in sorted(targets, key=lambda t: -t[1]):
        nc.cur_bb = nc.bb_map[bb.name]
        for sem, val in reversed(sem_vals):
            w = nc.engines[eng].wait_ge(sem, val)
            assert bb.instructions[-1].name == w.ins.name
            bb.instructions.pop()
            bb.instructions.insert(idx, w.ins)
        attached.append((str(eng), iname))
    nc.cur_bb = prev_bb
    assert attached, "no reader of the gather slots found"
    return attached


def _inject_wait_before_inst(nc, inst_name, sem, val):
    """Insert a standalone `wait sem >= val` immediately before the named
    instruction, on its engine's stream."""
    for bb in nc.m.functions[0].blocks:
        for idx, inst in enumerate(bb.instructions):
            if inst.name != inst_name:
                continue
            eng = inst.engine
            prev_bb = nc.cur_bb
            nc.cur_bb = nc.bb_map[bb.name]
            w = nc.engines[eng].wait_ge(sem, val)
            assert bb.instructions[-1].name == w.ins.name
            bb.instructions.pop()
            bb.instructions.insert(idx, w.ins)
            nc.cur_bb = prev_bb
            return True
    raise AssertionError(f"instruction {inst_name} not found")


def _inject_inc_after_inst(nc, inst_names, sem, val):
    """Insert `drain-then-inc(sem, val)` immediately after the LAST of the
    named instructions in scheduled order (engine-idle => the writers' stores
    have landed; compute instructions have a single HW update slot that Tile
    already occupies, so then_inc can't carry this)."""
    names = set(inst_names)
    best = None
    for bb in nc.m.functions[0].blocks:
        for idx, inst in enumerate(bb.instructions):
            if inst.name in names:
                best = (bb, idx, inst)
    assert best is not None, f"none of {inst_names} found"
    bb, idx, inst = best
    prev_bb = nc.cur_bb
    nc.cur_bb = nc.bb_map[bb.name]
    n_before = len(bb.instructions)
    nc.engines[inst.engine].maybe_drain_then_inc((sem, val), fusable=False)
    added = bb.instructions[n_before:]
    del bb.instructions[n_before:]
    for k, a in enumerate(added):
        bb.instructions.insert(idx + 1 + k, a)
    nc.cur_bb = prev_bb


def _build_module():
    nc = bacc.Bacc("TRN2", target_bir_lowering=False, debug=False,
                   enable_asserts=False, num_devices=NCORES,
                   num_swdge_queues=4)

    in_xa = nc.dram_tensor("in_xa", [128, NT * DA], F32, kind="ExternalInput").ap()
    in_ct = nc.dram_tensor("in_ct", [D, K], F32, kind="ExternalInput").ap()
    out_CT = nc.dram_tensor("out_ct", [D, K], F32, kind="ExternalOutput").ap()

    # launch synchronizer: a NEFF with a cross-core collective gets a
    # runtime-rendezvoused 8-core start (without one, per-core dispatch skew
    # is multi-ms).  It lives in its OWN block before the Tile block with
    # nothing waiting on its completion, so the cc stream's slow init
    # barrier + AllReduce (~87us) run concurrently and never gate the
    # engines' finish.  Input is uninitialized garbage - never read.
    dsy_i = nc.inline_tensor(np.zeros((1, 8), np.float32), "dsync_i")
    dsy_o = nc.dram_tensor("dsync_o", [1, 8], F32, kind="Internal",
                           addr_space="Shared")
    cc_sem = nc.alloc_semaphore("ccsync_sem")
    with nc.Block():
        nc.gpsimd.collective_compute(
            "AllReduce", ALU.add, replica_groups=[list(range(NCORES))],
            ins=[dsy_i.ap().opt()], outs=[dsy_o.ap().opt()]).then_inc(cc_sem, 1)

    with tile.TileContext(nc) as tc:
        arr_sems = [nc.alloc_semaphore(f"arr_sem{k}") for k in range(3)]
        loc_sems = [nc.alloc_semaphore(f"loc_sem{q}") for q in range(3)]
        gsem = nc.alloc_semaphore("gsb_ready_sem")
        with tc.tile_pool(name="per", bufs=1) as per, \
             tc.tile_pool(name="psg", bufs=1, space="PSUM") as psg, \
             tc.tile_pool(name="psa", bufs=1, space="PSUM") as psa, \
             tc.tile_pool(name="psb", bufs=1, space="PSUM") as psb, \
             tc.tile_pool(name="pso", bufs=1, space="PSUM") as pso:

            # ---------------- tiles ----------------
            Xc = [per.tile([128, tpc * DA], F32, name=f"xc{c}", tag=f"xc{c}")
                  for c, tpc in enumerate(CHUNKS)]
            CT65h = [per.tile([DA, 512], F32, name="ct65a", tag="ct65a"),
                     per.tile([DA, 512], F32, name="ct65b", tag="ct65b")]
            CTsq = per.tile([D, K], BF16, tag="ctsq")
            # hypercube reduce state: Bk = partial sum after k rounds (sent
            # in round k), Rk = remote landing slot for round k
            Bk = [per.tile([128, SLOT_W], F32, name=f"hcb{k}", tag=f"hcb{k}")
                  for k in range(3)]
            Rk = [per.tile([128, SLOT_W], F32, name=f"hcr{k}", tag=f"hcr{k}")
                  for k in range(3)]
            Gg = per.tile([DA, DA], F32, tag="gg")
            invmh = [per.tile([1, 512], F32, name="invma", tag="invma"),
                     per.tile([1, 512], F32, name="invmb", tag="invmb")]
            ivB = [per.tile([D, 512], F32, name="ivba", tag="ivba"),
                   per.tile([D, 512], F32, name="ivbb", tag="ivbb")]
            massh = [per.tile([1, 512], F32, name="massa", tag="massa"),
                     per.tile([1, 512], F32, name="massb", tag="massb")]
            sc1 = per.tile([1, 8], F32, tag="sc1")
            a_s = per.tile([1, 1], F32, tag="a_s")
            s2b = per.tile([DA, 1], F32, tag="s2b")
            ones64b = per.tile([D, 1], BF16, tag="ones64b")
            ones1 = per.tile([1, D], F32, tag="ones1")

            psG = psg.tile([DA, DA], F32, tag="psg")            # 1 bank
            pdA = psa.tile([1, K], F32, tag="pda")              # cc row
            pdBh = [psb.tile([D, 512], F32, name="pdba", tag="pdba"),
                    psb.tile([D, 512], F32, name="pdbb", tag="pdbb")]
            psOh = [pso.tile([DA, 512], F32, name="psoa", tag="psoa"),
                    pso.tile([DA, 512], F32, name="psob", tag="psob")]

            # ---------------- input DMA ----------------
            # chunk 0 issued first: the PE-bound G0 chain starts (and
            # therefore ends) earlier the sooner it lands
            offs = [0]
            for tpc in CHUNKS:
                offs.append(offs[-1] + tpc * DA)
            nc.sync.dma_start(Xc[0][:], in_xa[:, offs[0]:offs[1]])
            nc.sync.dma_start(Xc[1][:], in_xa[:, offs[1]:offs[2]])
            nc.sync.dma_start(CT65h[0][0:D, :], in_ct[:, 0:512])
            nc.sync.dma_start(CT65h[1][0:D, :], in_ct[:, 512:1024])
            for c in range(2, len(CHUNKS)):
                nc.sync.dma_start(Xc[c][:], in_xa[:, offs[c]:offs[c + 1]])
            nc.vector.memset(ones64b[:], 1.0)
            nc.vector.memset(ones1[:], 1.0)
            # send buffers fully defined (rows 65..127 ride along in every
            # frame; zeros keep the sim's finite-checks and peers' Rk clean)
            nc.vector.memset(Bk[0][:], 0.0)
            nc.vector.memset(Bk[1][:], 0.0)
            gsb_zero = nc.vector.memset(Bk[2][:], 0.0)

            # ---- XOR hypercube reduce frames: round k exchanges the
            # running partial with peer tpb^(2^k) and adds.  3 frames of Q7
            # descriptor generation (~6.9us each, THE serializer) instead of
            # 7 for the flat all-gather.  Each round has its own arrival
            # semaphore (rounds from different peers can land out of order)
            # and its own SWDGE queue so its trigger fires only its frame.
            # Emitted AFTER the memsets: the prep's Bk read must not give
            # the memsets WAR waits on the Pool stream (deadlocks against
            # the gsem-gated triggers sitting between the preps).
            trigs = []
            DELTAS = (4, 2, 1)   # cross-die (D2D) exchange first: it gets
                                 # the most slack before its data is needed
            for k in range(3):
                j = DELTAS[k]
                rd = [None] * 8
                rd[j] = (0, j)
                nc.gpsimd.remote_dma_broadcast(
                    out_ap=Rk[k][:], in_ap=Bk[k][:],
                    remote_sem=arr_sems[k], local_sem=loc_sems[k],
                    rdests=rd, queue_num=k)
                trigs.append(nc.gpsimd.trigger_dma(count=None, queue_num=k))

            # cc0 = colsum(C^2) in pdA row 0 (PE, before the G0 chain)
            nc.scalar.activation(CTsq[:, 0:512], CT65h[0][0:D, :], AF.Square)
            nc.scalar.activation(CTsq[:, 512:1024], CT65h[1][0:D, :], AF.Square)
            nc.tensor.matmul(pdA[0:1, 0:512], lhsT=ones64b[:],
                             rhs=CTsq[:, 0:512], start=True, stop=True)
            nc.tensor.matmul(pdA[0:1, 512:1024], lhsT=ones64b[:],
                             rhs=CTsq[:, 512:1024], start=True, stop=True)

            # ---- G0 = sum_t Xa_t^T Xa_t  (fp32 PSUM accumulation) ----
            for c, tpc in enumerate(CHUNKS):
                xa3 = Xc[c][:].rearrange("p (t e) -> p t e", e=DA)
                for t in range(tpc):
                    lhs = xa3[:, t, :]
                    nc.tensor.matmul(psG[:], lhsT=lhs, rhs=lhs,
                                     start=(c == 0 and t == 0),
                                     stop=(c == len(CHUNKS) - 1
                                           and t == tpc - 1))

            # ---- a = -gamma/(4*mc), local and replicated ----
            nc.vector.tensor_reduce(sc1[:, 0:1], pdA[0:1, 0:K], axis=AX.X,
                                    op=ALU.max)                       # mc
            nc.vector.reciprocal(sc1[:, 1:2], sc1[:, 0:1])
            nc.vector.tensor_scalar_mul(a_s[:], sc1[:, 1:2], -GAMMA / 4.0)
            nc.vector.tensor_scalar_mul(sc1[:, 2:3], sc1[:, 1:2], GAMMA / 2.0)

            # broadcast -2a to partitions 0..63 (PE)
            nc.tensor.matmul(pdBh[0][0:D, 0:1], lhsT=ones1[:], rhs=sc1[:, 2:3],
                             start=True, stop=True)
            nc.vector.memset(s2b[D:DA, :], 1.0)   # mass row unscaled
            nc.vector.tensor_copy(s2b[0:D, :], pdBh[0][0:D, 0:1])

            # mass row for iteration 1: 1 + a*cc0 (replicated)
            nc.scalar.activation(CT65h[0][D:DA, :], pdA[0:1, 0:512], AF.Copy,
                                 bias=1.0, scale=a_s[:])
            nc.scalar.activation(CT65h[1][D:DA, :], pdA[0:1, 512:1024], AF.Copy,
                                 bias=1.0, scale=a_s[:])

            # ---- scaled local partial [-2a*G0[0:64]; G0[64]] -> B0
            # (single 65-row activation; s2b row 64 = 1.0 keeps the mass
            # row unscaled)
            gsb_scale = nc.scalar.activation(Bk[0][0:DA, :], psG[0:DA, :],
                                             AF.Copy, scale=s2b[:])
            gsb_last = gsb_scale

            # ---- hypercube round adds (arrival waits injected
            # post-schedule): B1 = B0 + R0, B2 = B1 + R1, Gg = B2 + R2
            add1 = nc.vector.tensor_tensor(Bk[1][0:DA, :], Bk[0][0:DA, :],
                                           Rk[0][0:DA, :], op=ALU.add)
            add2 = nc.vector.tensor_tensor(Bk[2][0:DA, :], Bk[1][0:DA, :],
                                           Rk[1][0:DA, :], op=ALU.add)
            add3 = nc.vector.tensor_tensor(Gg[:], Bk[2][0:DA, :],
                                           Rk[2][0:DA, :], op=ALU.add)

            # ---------------- iterations ----------------
            # Two fixed-point iterations, software-pipelined in 512-column
            # halves with SEPARATE tiles per half (dependency tracking is
            # tile-granular, so shared tiles would serialize the halves).
            for h in range(2):                            # W1 = Gs @ rhs1
                nc.tensor.matmul(psOh[h][:], lhsT=Gg[:], rhs=CT65h[h][:],
                                 start=True, stop=True)
                nc.vector.tensor_copy(CT65h[h][:], psOh[h][:])   # rhs2 = W1
            for h in range(2):                            # W2 = Gs @ rhs2
                nc.tensor.matmul(psOh[h][:], lhsT=Gg[:], rhs=CT65h[h][:],
                                 start=True, stop=True)
                # mass staged to SBUF p0 (the custom DVE op misreads a PSUM
                # AP with a nonzero partition offset)
                nc.vector.tensor_copy(massh[h][:], psOh[h][D:DA, :])
                nc.vector.reciprocal_approx_fast(invmh[h][:], massh[h][:])
            for h in range(2):                            # C = W2[0:64]/W2[64]
                # gpsimd partition-broadcast of 1/mass to 64 partitions
                # (cheaper than the fp32 LOW_HIGH rank-1 PE matmul); the
                # divide reads W2 straight from PSUM (single PSUM operand)
                nc.gpsimd.partition_broadcast(ivB[h][:], invmh[h][:])
                nc.vector.tensor_mul(CT65h[h][0:D, :], psOh[h][0:D, :],
                                     ivB[h][:])
                nc.sync.dma_start(out_CT[:, 512 * h:512 * (h + 1)],
                                  CT65h[h][0:D, :])

    _dedupe_ldweights(nc)
    # per-round arrival gates (remote delivery is invisible to the
    # scheduler; these are the REAL cross-core synchronization).  Rounds 0/1
    # gate their adds; round 2 is read by BOTH the final add (DVE) and the
    # R2-part iter-1 matmul (PE ldweights), so gate each engine's first
    # hcr2 reader.
    adds = [add1, add2, add3]
    for k in range(3):
        _inject_wait_before_inst(nc, adds[k].ins.name, arr_sems[k], 2)
    # gsem chain: +1 after the send-buffer memsets, +1 after the scalar B0
    # writes, +1 after each round add; trigger k fires at gsem >= 2 + k
    _inject_inc_after_inst(nc, [gsb_zero.ins.name], gsem, 1)
    _inject_inc_after_inst(nc, [gsb_scale.ins.name, gsb_last.ins.name], gsem, 1)
    _inject_inc_after_inst(nc, [add1.ins.name], gsem, 1)
    _inject_inc_after_inst(nc, [add2.ins.name], gsem, 1)
    for k, t in enumerate(trigs):
        _inject_wait_before_inst(nc, t.ins.name, gsem, 2 + k)
    nc.finalize()
    _build_module.injected = [(f"round{k}", adds[k].ins.name)
                              for k in range(3)]
    return nc


def _dedupe_ldweights(nc):
    """Drop an InstLdweights whose weights AP equals the immediately
    preceding one in the scheduled PE stream (the HW keeps weights
    across matmuls)."""
    def sig(inst):
        a = inst.ins[0]
        try:
            return (a.memorylocation.name, a.offset, tuple(map(tuple, a.ap)))
        except Exception:
            return ("?", repr(a))

    removed = 0
    for bb in nc.m.functions[0].blocks:
        prev_sig = None
        keep = []
        for i in bb.instructions:
            if str(getattr(i, "engine", "")) == "EngineType.PE":
                tn = type(i).__name__
                if tn == "InstLdweights":
                    s = sig(i)
                    if s == prev_sig and not i.has_wait() and not i.has_update():
                        removed += 1
                        del nc.inst_map[i.name]
                        continue
                    prev_sig = s
                elif tn == "InstMatmult" and getattr(i, "is_transpose", False):
                    prev_sig = None
            keep.append(i)
        if removed:
            bb.instructions = keep
    return removed


_NC_CACHE = None


def _get_module():
    global _NC_CACHE
    if _NC_CACHE is None:
        _NC_CACHE = _build_module()
    return _NC_CACHE


def _marshal(X, clusters):
    X = np.ascontiguousarray(np.asarray(X, np.float32))
    C0 = np.ascontiguousarray(np.asarray(clusters, np.float32))
    CT0 = np.ascontiguousarray(C0.T)
    in_maps = []
    for c in range(NCORES):
        Xc = X[c * NL:(c + 1) * NL].reshape(128, NT, D)
        xa = np.empty((128, NT, DA), np.float32)
        xa[:, :, 0:D] = Xc
        xa[:, :, D] = 1.0
        in_maps.append({"in_xa": xa.reshape(128, NT * DA),
                        "in_ct": CT0})
    return in_maps


def kernel(X, clusters):
    nc = _get_module()
    in_maps = _marshal(X, clusters)
    trace = bool(int(os.environ.get("VQ_TRACE", "0")))
    last_err = None
    for attempt in range(2):
        try:
            res = bass_utils.run_bass_kernel_spmd(
                nc, [m.copy() for m in in_maps],
                core_ids=list(range(NCORES)), trace=trace)
            break
        except Exception as e:  # wedged device: retry once in-process
            last_err = e
            if attempt == 1:
                raise
    kernel.last_results = res
    ct = np.asarray(res.results[0]["out_ct"], np.float32)
    return np.ascontiguousarray(ct.T)
